# revision 9
# baseline (speedup 1.0000x reference)
"""Trainium2 Bass kernel for a Mamba-style selective-scan block.

Full computation (B=4, L=512, DM=1024, d_inner=2048, N=16, dt_rank=64, K=4):
    xz = x @ W_in.T ; xp, z = split(xz)
    u  = silu(causal_depthwise_conv(xp) + conv_b)
    x_dbl = u @ W_x.T -> (dlt, Bm, Cm)
    delta = softplus(dlt @ W_dt.T + b_dt)
    s_t = exp(delta*A)*s_{t-1} + delta*B_t*u_t ;  y_t = Cm_t . s_t
    out = ((y + u*D) * silu(z)) @ W_out.T

Sharding: 8 cores = 4 batch rows x 2 halves of d_inner. The x-branch
(conv/u) is replicated per core (x_dbl needs the full d_inner
contraction); everything else is sharded on d_inner. The host permutes
the d_inner axis per core so that "my half" is always tiles 0..7 - the
device program is identical on all cores (SPMD). out_proj partial sums
(contraction over d_inner halves) are summed on the host.

Device layout is [d on partitions, t on free]; the L-recurrence runs as
hardware tensor_tensor_scan ops on [128, 512] tiles, one per (d-tile, n).
"""

import ml_dtypes
import numpy as np

import concourse.bass as bass
import concourse.mybir as mybir
import concourse.tile as tile
from concourse import bacc
from concourse.bass_utils import run_bass_kernel_spmd

F32 = mybir.dt.float32
F32R = mybir.dt.float32r
BF16 = mybir.dt.bfloat16

B, L, DM = 4, 512, 1024
DI = 2048            # d_inner (full)
DH = 1024            # d_inner per core
N = 16               # ssm state size
RK = 64              # dt_rank
KC = 4               # conv kernel size
P = 128
KT = DM // P         # 8  k-tiles for in_proj
NTF = DI // P        # 16 d-tiles (full)
NTH = DH // P        # 8  d-tiles (half)
G = 2                # n-group size for the scan phase
NG = N // G          # 8  n-groups

mult = mybir.AluOpType.mult
add = mybir.AluOpType.add


def _build_program():
    nc = bacc.Bacc(num_devices=8)

    xt_d = nc.dram_tensor("xt", [DM, L], F32R, kind="ExternalInput")
    win_d = nc.dram_tensor("win", [DM, DI + DH], F32R, kind="ExternalInput")
    wx_d = nc.dram_tensor("wx", [DI, RK + 2 * N], BF16, kind="ExternalInput")
    wdt_d = nc.dram_tensor("wdt", [RK, DH], BF16, kind="ExternalInput")
    wout_d = nc.dram_tensor("wout", [DH, DM], BF16, kind="ExternalInput")
    convw_d = nc.dram_tensor("convw", [DI, KC], F32, kind="ExternalInput")
    convb_d = nc.dram_tensor("convb", [DI, 1], F32, kind="ExternalInput")
    amat_d = nc.dram_tensor("amat", [DH, N], F32, kind="ExternalInput")
    bdt_d = nc.dram_tensor("bdt", [DH, 1], F32, kind="ExternalInput")
    dpar_d = nc.dram_tensor("dpar", [DH, 1], F32, kind="ExternalInput")
    outp_d = nc.dram_tensor("outp", [DM, L], F32, kind="ExternalOutput")
    bcstage_d = nc.dram_tensor("bcstage", [2 * N, L], BF16)

    r32r = lambda ap: ap.bitcast(F32R)

    with tile.TileContext(nc) as tc:
        with (
            tc.tile_pool(name="consts", bufs=1) as cpool,
            tc.tile_pool(name="xt", bufs=KT) as xtp,
            tc.tile_pool(name="win", bufs=12) as winp,
            tc.tile_pool(name="xp", bufs=3) as xpp,
            tc.tile_pool(name="ctmp", bufs=4) as ctp,
            tc.tile_pool(name="u", bufs=NTF) as up,
            tc.tile_pool(name="zs", bufs=NTH) as zp,
            tc.tile_pool(name="wx", bufs=NTF) as wxp,
            tc.tile_pool(name="xdbl", bufs=1) as xdp,
            tc.tile_pool(name="delta", bufs=NTH) as dp,
            tc.tile_pool(name="wd", bufs=NTH) as wdp,
            tc.tile_pool(name="bc", bufs=2) as bcp,
            tc.tile_pool(name="stk", bufs=2) as stkp,
            tc.tile_pool(name="y", bufs=NTH) as yp,
            tc.tile_pool(name="wout", bufs=12) as woutp,
            tc.tile_pool(name="osb", bufs=2) as op_,
            tc.tile_pool(name="ps", bufs=4, space="PSUM") as psp,
        ):
            # ---- constant loads ----
            convw_t = cpool.tile([P, NTF, KC], F32)
            nc.sync.dma_start(out=convw_t, in_=convw_d.ap().rearrange("(j p) k -> p j k", p=P))
            convb_t = cpool.tile([P, NTF], F32)
            nc.sync.dma_start(out=convb_t, in_=convb_d.ap().rearrange("(j p) o -> p (j o)", p=P))
            amat_t = cpool.tile([P, NTH, N], F32)
            nc.sync.dma_start(out=amat_t, in_=amat_d.ap().rearrange("(j p) n -> p j n", p=P))
            bdt_t = cpool.tile([P, NTH], F32)
            nc.sync.dma_start(out=bdt_t, in_=bdt_d.ap().rearrange("(j p) o -> p (j o)", p=P))
            dpar_t = cpool.tile([P, NTH], F32)
            nc.sync.dma_start(out=dpar_t, in_=dpar_d.ap().rearrange("(j p) o -> p (j o)", p=P))
            wdt_t = cpool.tile([RK, DH], BF16)
            nc.sync.dma_start(out=wdt_t, in_=wdt_d.ap())

            xt_t = []
            for k in range(KT):
                t = xtp.tile([P, L], F32R, tag="xt")
                nc.sync.dma_start(out=t, in_=xt_d.ap()[k * P:(k + 1) * P, :])
                xt_t.append(t)

            wx_t = []
            for j in range(NTF):
                t = wxp.tile([P, RK + 2 * N], BF16, tag="wx")
                nc.sync.dma_start(out=t, in_=wx_d.ap()[j * P:(j + 1) * P, :])
                wx_t.append(t)

            win_r = win_d.ap().rearrange("(k p) m -> p k m", p=P)

            # ---- phase 1: in_proj + conv + silu ----
            u_t = []
            zs_t = []
            for m in range(NTF + NTH):
                ps = psp.tile([P, L], F32, tag="mm")
                for k in range(KT):
                    kt = winp.tile([P, P], F32R, tag="win")
                    # gpsimd SWDGE: slot-reuse DMAs need >1 sync wait, which
                    # HWDGE descriptors cannot encode
                    nc.gpsimd.dma_start(out=kt, in_=win_r[:, k, m * P:(m + 1) * P])
                    nc.tensor.matmul(ps, lhsT=kt, rhs=xt_t[k],
                                     start=(k == 0), stop=(k == KT - 1))
                if m < NTF:
                    j = m
                    xp_t = xpp.tile([P, L + KC - 1], F32, tag="xp")
                    nc.vector.memset(xp_t[:, 0:KC - 1], 0.0)
                    nc.scalar.copy(xp_t[:, KC - 1:KC - 1 + L], ps)
                    acc = ctp.tile([P, L], F32, tag="c")
                    nc.vector.tensor_scalar(acc, xp_t[:, 0:L], convw_t[:, j, 0:1],
                                            convb_t[:, j:j + 1], mult, add)
                    for k in range(1, KC):
                        acc2 = ctp.tile([P, L], F32, tag="c")
                        nc.vector.scalar_tensor_tensor(acc2, xp_t[:, k:k + L],
                                                       convw_t[:, j, k:k + 1], acc,
                                                       mult, add)
                        acc = acc2
                    ut = up.tile([P, L], BF16, tag="u")
                    nc.scalar.activation(ut, acc, mybir.ActivationFunctionType.Silu)
                    u_t.append(ut)
                else:
                    zt = zp.tile([P, L], BF16, tag="zs")
                    nc.scalar.activation(zt, ps, mybir.ActivationFunctionType.Silu)
                    zs_t.append(zt)

            # ---- phase 2: x_dbl = u @ W_x.T  -> [96, 512] ----
            xdbl_ps = psp.tile([RK + 2 * N, L], F32, tag="mm")
            for j in range(NTF):
                nc.tensor.matmul(xdbl_ps, lhsT=wx_t[j], rhs=u_t[j],
                                 start=(j == 0), stop=(j == NTF - 1))
            xdbl_sb = xdp.tile([RK + 2 * N, L], BF16, tag="xdbl")
            nc.scalar.copy(xdbl_sb, xdbl_ps)

            # stage B/C rows to DRAM for the partition-broadcast loads
            nc.gpsimd.dma_start(out=bcstage_d.ap(), in_=xdbl_sb[RK:RK + 2 * N, :])

            # ---- phase 3: delta = softplus(dlt @ W_dt.T + b_dt); wd = delta*u ----
            delta_t = []
            wd_t = []
            for j in range(NTH):
                ps = psp.tile([P, L], F32, tag="mm")
                nc.tensor.matmul(ps, lhsT=wdt_t[:, j * P:(j + 1) * P],
                                 rhs=xdbl_sb[0:RK, :], start=True, stop=True)
                # softplus(x) = ln(1 + exp(x)) - Softplus has no ACT table set,
                # but exp and ln share one (natural_log_exp_and_others).
                et = dp.tile([P, L], F32, tag="dexp", bufs=2)
                nc.scalar.activation(et, ps, mybir.ActivationFunctionType.Exp,
                                     bias=bdt_t[:, j:j + 1])
                dt_ = dp.tile([P, L], F32, tag="delta")
                nc.scalar.activation(dt_, et, mybir.ActivationFunctionType.Ln,
                                     bias=1.0)
                delta_t.append(dt_)
                wdt_j = wdp.tile([P, L], BF16, tag="wd")
                nc.vector.tensor_tensor(wdt_j, dt_, u_t[j], mult)
                wd_t.append(wdt_j)

            # ---- y init: y_j = u_j * D ----
            y_t = []
            for j in range(NTH):
                yt = yp.tile([P, L], BF16, tag="y")
                nc.vector.tensor_scalar_mul(yt, u_t[j], dpar_t[:, j:j + 1])
                y_t.append(yt)

            # ---- phase 4: selective scan, grouped over n ----
            for g in range(NG):
                bg = bcp.tile([P, G, L], BF16, tag="bgrp")
                nc.gpsimd.dma_start(
                    out=bg, in_=bcstage_d.ap()[g * G:(g + 1) * G, :].partition_broadcast(P))
                cg = bcp.tile([P, G, L], BF16, tag="cgrp")
                nc.gpsimd.dma_start(
                    out=cg, in_=bcstage_d.ap()[N + g * G:N + (g + 1) * G, :].partition_broadcast(P))
                for j in range(NTH):
                    ag = stkp.tile([P, G * L], F32, tag="a")
                    for i in range(G):
                        nc.scalar.activation(ag[:, i * L:(i + 1) * L], delta_t[j],
                                             mybir.ActivationFunctionType.Exp,
                                             scale=amat_t[:, j, g * G + i:g * G + i + 1])
                    bgm = stkp.tile([P, G, L], BF16, tag="b")
                    wd_bc = wd_t[j][:, None, :].to_broadcast([P, G, L])
                    nc.vector.tensor_tensor(bgm, wd_bc, bg, mult)
                    sg = stkp.tile([P, G * L], BF16, tag="s")
                    for i in range(G):
                        nc.vector.tensor_tensor_scan(
                            sg[:, i * L:(i + 1) * L], ag[:, i * L:(i + 1) * L],
                            bgm[:, i, :], 0.0, mult, add)
                    ym = stkp.tile([P, G * L], BF16, tag="ym")
                    nc.vector.tensor_tensor(ym, sg, cg.rearrange("p g l -> p (g l)"), mult)
                    for i in range(G):
                        nc.vector.tensor_tensor(y_t[j], y_t[j], ym[:, i * L:(i + 1) * L], add)

            # ---- phase 5: gate ----
            yg_t = []
            for j in range(NTH):
                yg = up.tile([P, L], BF16, tag="u")
                nc.vector.tensor_tensor(yg, y_t[j], zs_t[j], mult)
                yg_t.append(yg)

            # ---- phase 6: out_proj (partial over this d half) ----
            wout_r = wout_d.ap().rearrange("(k p) m -> p k m", p=P)
            for m in range(KT):
                ps = psp.tile([P, L], F32, tag="mm")
                for k in range(NTH):
                    kt = woutp.tile([P, P], BF16, tag="wout")
                    nc.gpsimd.dma_start(out=kt, in_=wout_r[:, k, m * P:(m + 1) * P])
                    nc.tensor.matmul(ps, lhsT=kt, rhs=yg_t[k],
                                     start=(k == 0), stop=(k == NTH - 1))
                osb = op_.tile([P, L], F32, tag="osb")
                nc.scalar.copy(osb, ps)
                nc.gpsimd.dma_start(out=outp_d.ap()[m * P:(m + 1) * P, :], in_=osb)

    nc.compile()
    return nc


_PROG = None


def _prep_core_inputs(inputs):
    x = np.asarray(inputs["x"], np.float32)
    W_in = np.asarray(inputs["W_in"], np.float32)
    conv_w = np.asarray(inputs["conv_w"], np.float32)
    conv_b = np.asarray(inputs["conv_b"], np.float32)
    W_x = np.asarray(inputs["W_x"], np.float32)
    W_dt = np.asarray(inputs["W_dt"], np.float32)
    b_dt = np.asarray(inputs["b_dt"], np.float32)
    A_log = np.asarray(inputs["A_log"], np.float32)
    D_param = np.asarray(inputs["D_param"], np.float32)
    W_out = np.asarray(inputs["W_out"], np.float32)

    A = -np.exp(A_log)
    half_maps = []
    for h in (0, 1):
        sl = slice(h * DH, (h + 1) * DH)
        perm = np.concatenate([np.arange(h * DH, (h + 1) * DH),
                               np.arange((1 - h) * DH, (1 - h) * DH + DH)])
        half_maps.append({
            "win": np.ascontiguousarray(np.concatenate(
                [W_in[:DI][perm].T, W_in[DI + h * DH:DI + (h + 1) * DH].T], axis=1)),
            "wx": np.ascontiguousarray(W_x.T[perm]).astype(ml_dtypes.bfloat16),
            "wdt": np.ascontiguousarray(W_dt[sl].T).astype(ml_dtypes.bfloat16),
            "wout": np.ascontiguousarray(W_out[:, sl].T).astype(ml_dtypes.bfloat16),
            "convw": np.ascontiguousarray(conv_w[perm, 0, :]),
            "convb": np.ascontiguousarray(conv_b[perm].reshape(DI, 1)),
            "amat": np.ascontiguousarray(A[sl]),
            "bdt": np.ascontiguousarray(b_dt[sl].reshape(DH, 1)),
            "dpar": np.ascontiguousarray(D_param[sl].reshape(DH, 1)),
        })

    in_maps = []
    for b in range(B):
        xt = np.ascontiguousarray(x[b].T)
        for h in (0, 1):
            in_maps.append({"xt": xt, **half_maps[h]})
    return in_maps


def kernel(**inputs):
    global _PROG
    if _PROG is None:
        _PROG = _build_program()
    in_maps = _prep_core_inputs(inputs)
    res = run_bass_kernel_spmd(_PROG, in_maps, list(range(8)))
    out = np.empty((B, L, DM), np.float32)
    for b in range(B):
        part = res.results[2 * b]["outp"] + res.results[2 * b + 1]["outp"]
        out[b] = part.T
    return out


if __name__ == "__main__":
    rng = np.random.default_rng(0)
    fake = {
        "x": rng.standard_normal((B, L, DM), dtype=np.float32),
        "W_in": rng.standard_normal((2 * DI, DM), dtype=np.float32) * 0.02,
        "conv_w": rng.standard_normal((DI, 1, KC), dtype=np.float32) * 0.3,
        "conv_b": np.zeros((DI,), np.float32),
        "W_x": rng.standard_normal((RK + 2 * N, DI), dtype=np.float32) * 0.02,
        "W_dt": rng.standard_normal((DI, RK), dtype=np.float32) * 0.125,
        "b_dt": rng.standard_normal((DI,), dtype=np.float32) - 4.0,
        "A_log": np.broadcast_to(np.log(np.arange(1, N + 1, dtype=np.float32)),
                                 (DI, N)).copy(),
        "D_param": np.ones((DI,), np.float32),
        "W_out": rng.standard_normal((DM, DI), dtype=np.float32) * 0.02,
    }
    out = kernel(**fake)
    print("kernel out", out.shape, out.dtype, np.abs(out).max())
